# revision 43
# baseline (speedup 1.0000x reference)
"""GQA attention kernel for 8 TRN2 NeuronCores.

Sharding: data-parallel over batch (B=2) x tensor-parallel over heads (4-way).
Core i handles batch i//4 and head-shard i%4 (8 query heads = 2 KV groups).
Out-proj is row-sharded; the 4 partial [S,D] outputs per batch are summed on
the host (cheap unshard step), bo added once.

Device kernel (per core, all bf16 matmuls, f32 PSUM):
  QT = Wq_sh.T @ xT          [512, S]   (x pre-transposed on host)
  KT = Wk_sh.T @ kvT         [128, S]  -> stored zero-padded per group so QK
  runs with K=128 (full PE-array tile mode; avoids the 64-row mode-switch
  weight-load penalty)
  V  = kvT.T  @ Wv_sh        [S, 128] -> per-group V_aug [S, 64+1] (ones col)
  per (head-pair (hc, hc+4), q-chunk 512): scores^T chunks [128 keys, 512 q]
  for both heads off one shared rhs -> exp (no max subtraction; scores are
  O(1)) -> causal mask via sliding window of a precomputed [128,1024] 0/1
  mask -> PV accumulate with ones-row giving softmax sums in row 64 ->
  normalize both heads with ONE selector matmul broadcasting the two
  reciprocal rows -> OT [512, S] -> out_partial = OT.T @ Wo_sh  [S, D] f32.
"""

import numpy as np

B, S, D = 2, 2048, 2048
H, G, HD, GS = 32, 8, 64, 4
HPC = 8     # query heads per core
GPC = 2     # kv groups per core
NCORES = 8
SCALE = 0.125  # 1/sqrt(64)

_CACHE = {}


def _build():
    import concourse.bass as bass
    import concourse.tile as tile
    from concourse import bacc, mybir

    f32 = mybir.dt.float32
    bf16 = mybir.dt.bfloat16
    AF = mybir.ActivationFunctionType
    ALU = mybir.AluOpType

    nc = bacc.Bacc("TRN2", target_bir_lowering=False, debug=False,
                   num_devices=NCORES)

    xT_d = nc.declare_dram_parameter("xT", [D, S], bf16, isOutput=False)
    kvT_d = nc.declare_dram_parameter("kvT", [D, S], bf16, isOutput=False)
    wq_d = nc.declare_dram_parameter("wq", [D, 512], bf16, isOutput=False)
    wk_d = nc.declare_dram_parameter("wk", [D, 128], bf16, isOutput=False)
    wv_d = nc.declare_dram_parameter("wv", [D, 128], bf16, isOutput=False)
    wo_d = nc.declare_dram_parameter("wo", [512, D], bf16, isOutput=False)
    bq_d = nc.declare_dram_parameter("bq", [128, 4], f32, isOutput=False)
    bk_d = nc.declare_dram_parameter("bk", [128, 1], f32, isOutput=False)
    bvt_d = nc.declare_dram_parameter("bvt", [128, 2 * 64], f32, isOutput=False)
    m0_d = nc.declare_dram_parameter("m0", [128, 1024], bf16, isOutput=False)
    # partials are summed across 4 cores on the host in f32; bf16 partials
    # halve the output DMA traffic (the structural tail) for ~1e-3 extra err
    out_d = nc.declare_dram_parameter("out", [S, D], bf16, isOutput=True)

    with tile.TileContext(nc) as tc:
        with (
            tc.tile_pool(name="persist", bufs=1) as persist,
            tc.tile_pool(name="stream", bufs=4) as stream,
            tc.tile_pool(name="small", bufs=3) as small,
            tc.tile_pool(name="probs", bufs=8) as probs_pool,
            tc.tile_pool(name="ps_s", bufs=3, space="PSUM") as ps_s,
            tc.tile_pool(name="ps_proj", bufs=2, space="PSUM") as ps_proj,
            tc.tile_pool(name="ps_o", bufs=3, space="PSUM") as ps_o,
        ):
            # ---- resident weight tiles (DMAs emitted in compute order) ----
            wq_sb = persist.tile([128, 16 * 512], bf16, tag="wq")   # chunk c at c*512
            wk_sb = persist.tile([128, 16 * 128], bf16, tag="wk")
            wv_sb = persist.tile([128, 16 * 128], bf16, tag="wv")
            wo_sb = persist.tile([128, 4 * 2048], bf16, tag="wo")
            m0_sb = persist.tile([128, 1024], bf16, tag="m0")
            bq_sb = persist.tile([128, 4], f32, tag="bq")
            bk_sb = persist.tile([128, 1], f32, tag="bk")
            bvt_sb = persist.tile([128, 2 * 64], f32, tag="bvt")

            warm_sb = persist.tile([128, 128], bf16, tag="warm")
            nc.vector.memset(warm_sb[:], 0.25)
            for w in range(16):
                wps = ps_s.tile([128, 128], f32, tag="sps", name="wps")
                nc.tensor.matmul(wps[:], lhsT=warm_sb[:], rhs=warm_sb[:],
                                 start=True, stop=True)

            # programming a DMA costs ~650ns of serial Sync-engine time, but
            # the FIRST pieces gate the first matmuls: tiny head, fat tail
            for lo, hi in ((0, 1), (1, 16)):
                nc.sync.dma_start(
                    out=wk_sb[:, lo * 128:hi * 128]
                    .rearrange("p (c j) -> p c j", j=128),
                    in_=wk_d[lo * 128:hi * 128, :]
                    .rearrange("(c p) j -> p c j", p=128))

            sel_sb = persist.tile([128, 128], bf16, tag="sel")
            rsb2a = persist.tile([128, 512], bf16, tag="rsb2a")
            rsb2b = persist.tile([128, 512], bf16, tag="rsb2b")

            # ---- resident projection outputs ----
            qt_sb = persist.tile([128, 4 * 2048], bf16, tag="qt")   # chunk hc at hc*2048
            # K^T stored twice, zero-padded per group: ktp0 rows 0:64 = group0
            # K dims (rows 64:128 zero); ktp1 rows 64:128 = group1 (rows 0:64
            # zero). QK then contracts K=128 against the full qt partition
            # range; the zero rows kill the other head's contribution.
            ktp0_sb = persist.tile([128, S], bf16, tag="ktp0")
            ktp1_sb = persist.tile([128, S], bf16, tag="ktp1")
            vaug_sb = persist.tile([128, 2 * 16 * 65], bf16, tag="vaug")  # [gl*1040+tok*65]
            ot_sb = persist.tile([128, 4 * 2048], bf16, tag="ot")

            nc.vector.memset(ktp0_sb[64:128, :], 0.0)
            nc.vector.memset(ktp1_sb[0:64, :], 0.0)
            # selector for the paired reciprocal broadcast: out rows 0:64 get
            # rhs row 0, out rows 64:128 get rhs row 64
            nc.vector.memset(sel_sb[:], 0.0)
            nc.vector.memset(sel_sb[0:1, 0:64], 1.0)
            nc.vector.memset(sel_sb[64:65, 64:128], 1.0)
            # only rows 0 and 64 are ever written after this -> no garbage
            nc.vector.memset(rsb2a[:], 0.0)
            nc.vector.memset(rsb2b[:], 0.0)

            # ---- chain emitters (as thunk lists for PE-filler interleave) ----
            def kv_chain_thunks(tch):
                """K/V projection for kv token chunk tch.

                Returns (dma_fn, compute_thunks): the DMA is emitted eagerly
                (a block ahead) so filler matmuls never starve on it."""
                th = []
                state = {}

                def dma():
                    kvt = stream.tile([128, 16 * 512], bf16, tag="xs", name="kvt")
                    pieces = ((0, 1), (1, 4), (4, 16)) if tch == 0 else \
                             ((0, 8), (8, 16))
                    for lo, hi in pieces:
                        nc.sync.dma_start(
                            out=kvt[:, lo * 512:hi * 512]
                            .rearrange("p (c j) -> p c j", j=512),
                            in_=kvT_d[lo * 128:hi * 128,
                                      tch * 512:(tch + 1) * 512]
                            .rearrange("(c p) j -> p c j", p=128))
                    if tch == 0:
                        nc.sync.dma_start(out=bk_sb[:], in_=bk_d[:, :])
                        nc.sync.dma_start(out=bvt_sb[:], in_=bvt_d[:, :])
                    state["kvt"] = kvt

                def kmm(c):
                    if c == 0:
                        state["kps"] = ps_proj.tile([128, 512], f32, tag="proj",
                                                    name="kps")
                    nc.tensor.matmul(
                        state["kps"][:], lhsT=wk_sb[:, c * 128:(c + 1) * 128],
                        rhs=state["kvt"][:, c * 512:(c + 1) * 512],
                        start=(c == 0), stop=(c == 15))
                    if c == 15:
                        win = slice(tch * 512, (tch + 1) * 512)
                        nc.vector.tensor_scalar(
                            ktp0_sb[0:64, win], state["kps"][0:64, :],
                            bk_sb[0:64, 0:1], None, ALU.add)
                        nc.vector.tensor_scalar(
                            ktp1_sb[64:128, win], state["kps"][64:128, :],
                            bk_sb[64:128, 0:1], None, ALU.add)
                for c in range(16):
                    th.append(lambda c=c: kmm(c))

                def vmm(tt, c):
                    if c == 0:
                        state["vps"] = ps_proj.tile([128, 128], f32, tag="proj",
                                                    name="vps")
                    nc.tensor.matmul(
                        state["vps"][:],
                        lhsT=state["kvt"][:, c * 512 + tt * 128:
                                          c * 512 + (tt + 1) * 128],
                        rhs=wv_sb[:, c * 128:(c + 1) * 128],
                        start=(c == 0), stop=(c == 15))
                    if c == 15:
                        tok = tch * 4 + tt
                        for gl in range(2):
                            base = gl * 1040 + tok * 65
                            nc.vector.tensor_tensor(
                                vaug_sb[:, base:base + 64],
                                state["vps"][:, gl * 64:(gl + 1) * 64],
                                bvt_sb[:, gl * 64:(gl + 1) * 64], ALU.add)
                            nc.vector.memset(
                                vaug_sb[:, base + 64:base + 65], 1.0)
                for tt in range(4):
                    for c in range(0, 16, 4):
                        # 4 small matmuls per thunk (they are ~68ns each)
                        def v4(tt=tt, c0=c):
                            for c in range(c0, c0 + 4):
                                vmm(tt, c)
                        th.append(v4)
                return dma, th

            def q_chain_thunks(qch, xt_tile=None):
                """Q projection for q chunk qch: (dma_fn, compute thunks)."""
                th = []
                state = {}

                if xt_tile is not None:
                    state["xt"] = xt_tile
                    dma = None
                else:
                    def dma():
                        xt = stream.tile([128, 16 * 512], bf16, tag="xs",
                                         name="xt")
                        nc.sync.dma_start(
                            out=xt.rearrange("p (c j) -> p c j", j=512),
                            in_=xT_d[:, qch * 512:(qch + 1) * 512]
                            .rearrange("(c p) j -> p c j", p=128))
                        state["xt"] = xt

                def qmm(hc, c):
                    if c == 0:
                        state["qps"] = ps_proj.tile([128, 512], f32, tag="proj",
                                                    name="qps")
                    nc.tensor.matmul(
                        state["qps"][:],
                        lhsT=wq_sb[:, c * 512 + hc * 128:c * 512 + (hc + 1) * 128],
                        rhs=state["xt"][:, c * 512:(c + 1) * 512],
                        start=(c == 0), stop=(c == 15))
                    if c == 15:
                        nc.vector.tensor_scalar(
                            qt_sb[:, hc * 2048 + qch * 512:
                                  hc * 2048 + (qch + 1) * 512],
                            state["qps"][:], bq_sb[:, hc:hc + 1], None, ALU.add)
                for hc in range(4):
                    for c in range(16):
                        th.append(lambda hc=hc, c=c: qmm(hc, c))
                return dma, th

            def outproj_thunks(jqb):
                """Out-projection for q block jqb (4 q-tiles x 4 col-chunks)."""
                th = []
                state = {}

                def omm(qt_i, cc, c):
                    if c == 0:
                        state["outp"] = ps_proj.tile([128, 512], f32, tag="proj",
                                                     name="outp")
                    nc.tensor.matmul(
                        state["outp"][:],
                        lhsT=ot_sb[:, c * 2048 + qt_i * 128:
                                   c * 2048 + (qt_i + 1) * 128],
                        rhs=wo_sb[:, c * 2048 + cc * 512:c * 2048 + (cc + 1) * 512],
                        start=(c == 0), stop=(c == 3))
                    if c == 3:
                        if cc == 0:
                            state["osb"] = stream.tile([128, 2048], bf16,
                                                       tag="osb", name="osb")
                        # scalar engine: idle outside the exp stream
                        nc.scalar.activation(
                            state["osb"][:, cc * 512:(cc + 1) * 512],
                            state["outp"][:], AF.Copy)
                        if cc == 3:
                            nc.sync.dma_start(
                                out=out_d[qt_i * 128:(qt_i + 1) * 128, :],
                                in_=state["osb"][:])
                for qt_i in range(jqb * 4, jqb * 4 + 4):
                    for cc in range(4):
                        for c in range(4):
                            th.append(lambda q=qt_i, cc=cc, c=c: omm(q, cc, c))
                return th

            # ---- filler queue machinery ----
            fillers = []
            fpos = [0]

            def pop_filler(n=1):
                while n > 0 and fpos[0] < len(fillers):
                    fillers[fpos[0]]()
                    fpos[0] += 1
                    n -= 1

            def drain_fillers_through(idx):
                while fpos[0] <= idx:
                    fillers[fpos[0]]()
                    fpos[0] += 1

            def qk0_pair(hc, jq):
                # next pair's first QK scores, pre-emitted before the current
                # pair's final PV so exp(0) overlaps the norm chain
                qbase = hc * 2048 + jq * 512
                rhs = qt_sb[:, qbase:qbase + 512]
                spsA = ps_s.tile([128, 512], f32, tag="sps", name="spsA")
                nc.tensor.matmul(spsA[:], lhsT=ktp0_sb[:, 0:128], rhs=rhs,
                                 start=True, stop=True)
                spsB = ps_s.tile([128, 512], f32, tag="sps", name="spsB")
                nc.tensor.matmul(spsB[:], lhsT=ktp1_sb[:, 0:128], rhs=rhs,
                                 start=True, stop=True)
                return (spsA, spsB, 0)

            # ---- attention for one head-pair (hc, hc+4) on one q-chunk ----
            def attention_pair(hc, jq, qk0=None):
                nkc = 4 * jq + 4
                qbase = hc * 2048 + jq * 512
                opsA = ps_o.tile([65, 512], f32, tag="ops", name="opsA")
                opsB = ps_o.tile([65, 512], f32, tag="ops", name="opsB")
                sps_t = {}
                pt_t = {}

                def emit_qk(kci):
                    m = max(0, kci * 128 - jq * 512)
                    rhs = qt_sb[:, qbase + m:qbase + 512]
                    spsA = ps_s.tile([128, 512], f32, tag="sps", name="spsA")
                    nc.tensor.matmul(
                        spsA[:, m:512],
                        lhsT=ktp0_sb[:, kci * 128:(kci + 1) * 128],
                        rhs=rhs, start=True, stop=True)
                    spsB = ps_s.tile([128, 512], f32, tag="sps", name="spsB")
                    nc.tensor.matmul(
                        spsB[:, m:512],
                        lhsT=ktp1_sb[:, kci * 128:(kci + 1) * 128],
                        rhs=rhs, start=True, stop=True)
                    sps_t[kci] = (spsA, spsB, m)

                def emit_exp(kci):
                    spsA, spsB, m = sps_t.pop(kci)
                    ptA = probs_pool.tile([128, 512], bf16, tag="pt", name="ptA")
                    nc.scalar.activation(ptA[:, m:512], spsA[:, m:512],
                                         AF.Exp, scale=SCALE)
                    ptB = probs_pool.tile([128, 512], bf16, tag="pt", name="ptB")
                    nc.scalar.activation(ptB[:, m:512], spsB[:, m:512],
                                         AF.Exp, scale=SCALE)
                    if kci >= 4 * jq:   # diagonal chunk -> mask
                        nc.vector.tensor_tensor(
                            ptA[:, m:512], ptA[:, m:512],
                            m0_sb[:, 512:1024 - m], ALU.mult)
                        nc.vector.tensor_tensor(
                            ptB[:, m:512], ptB[:, m:512],
                            m0_sb[:, 512:1024 - m], ALU.mult)
                    pt_t[kci] = (ptA, ptB, m)

                def emit_pv(kci):
                    ptA, ptB, m = pt_t.pop(kci)
                    nc.tensor.matmul(
                        opsA[:, m:512],
                        lhsT=vaug_sb[:, kci * 65:kci * 65 + 65],
                        rhs=ptA[:, m:512],
                        start=(kci == 0), stop=(kci == nkc - 1))
                    nc.tensor.matmul(
                        opsB[:, m:512],
                        lhsT=vaug_sb[:, 1040 + kci * 65:1040 + kci * 65 + 65],
                        rhs=ptB[:, m:512],
                        start=(kci == 0), stop=(kci == nkc - 1))

                rss2 = small.tile([33, 512], f32, tag="rss", name="rss2")

                def emit_pv_last(kci):
                    # final PV pair: tuck each head's sums copy right behind
                    # its matmul so the vector norm chain starts early
                    ptA, ptB, m = pt_t.pop(kci)
                    nc.tensor.matmul(
                        opsA[:, m:512],
                        lhsT=vaug_sb[:, kci * 65:kci * 65 + 65],
                        rhs=ptA[:, m:512], start=(kci == 0), stop=True)
                    nc.vector.tensor_copy(rss2[0:1, :], opsA[64:65, :])
                    nc.tensor.matmul(
                        opsB[:, m:512],
                        lhsT=vaug_sb[:, 1040 + kci * 65:1040 + kci * 65 + 65],
                        rhs=ptB[:, m:512], start=(kci == 0), stop=True)
                    nc.vector.tensor_copy(rss2[32:33, :], opsB[64:65, :])

                if qk0 is None:
                    emit_qk(0)
                else:
                    sps_t[0] = qk0
                nxt = None
                for kci in range(nkc):
                    emit_exp(kci)
                    if kci + 1 < nkc:
                        emit_qk(kci + 1)
                    pop_filler(3 if jq <= 1 else 2)
                    if kci < nkc - 1:
                        emit_pv(kci)
                    else:
                        if hc < 3:
                            nxt = qk0_pair(hc + 1, jq)
                        emit_pv_last(kci)

                # paired normalize: two reciprocal rows -> one K=128 selector
                # matmul broadcasts row 0 to out rows 0:64, row 64 to 64:128
                rsb2 = rsb2a if hc % 2 == 0 else rsb2b
                rs2 = small.tile([33, 512], f32, tag="rs", name="rs2")
                nc.vector.reciprocal_approx_fast(rs2[:], rss2[:])
                nc.vector.tensor_copy(rsb2[0:1, :], rs2[0:1, :])
                nc.vector.tensor_copy(rsb2[64:65, :], rs2[32:33, :])
                if hc == 3 and jq == 3:
                    # keep ps_proj free for the trailing out-proj chains:
                    # the sps pool is idle once the last scores are consumed
                    bps = ps_s.tile([128, 512], f32, tag="sps", name="bps")
                else:
                    bps = ps_proj.tile([128, 512], f32, tag="proj", name="bps")
                nc.tensor.matmul(bps[:], lhsT=sel_sb[:], rhs=rsb2[:],
                                 start=True, stop=True)
                bsb = small.tile([128, 512], f32, tag="bsb", name="bsb")
                nc.vector.tensor_copy(bsb[:], bps[:])
                nc.vector.tensor_tensor(
                    ot_sb[0:64, qbase:qbase + 512],
                    opsA[0:64, :], bsb[0:64, :], ALU.mult)
                nc.vector.tensor_tensor(
                    ot_sb[64:128, qbase:qbase + 512],
                    opsB[0:64, :], bsb[64:128, :], ALU.mult)
                return nxt

            # ---- emission schedule ----
            # prologue: KV(0) K-chain first (gated only on wk piece 0 + kvt
            # piece 0), then Q(0); remaining weight DMAs stream in behind
            kv0_dma, kv0 = kv_chain_thunks(0)
            kv0_dma()                     # kvt0 DMA pieces + bk + bvt
            for t in kv0[:16]:            # 16 K matmuls
                t()
            nc.sync.dma_start(out=bq_sb[:], in_=bq_d[:, :])
            nc.sync.dma_start(
                out=wv_sb.rearrange("p (c j) -> p c j", j=128),
                in_=wv_d.rearrange("(c p) j -> p c j", p=128))
            nc.sync.dma_start(out=m0_sb[:], in_=m0_d[:, :])
            # xt0 + wq interleaved in 2 pieces each: Q chain c=0 needs only
            # piece 0 of both
            xt0 = stream.tile([128, 16 * 512], bf16, tag="xs", name="xt0")
            for c4 in range(0, 16, 4):
                nc.sync.dma_start(
                    out=xt0[:, c4 * 512:(c4 + 4) * 512]
                    .rearrange("p (c j) -> p c j", j=512),
                    in_=xT_d[c4 * 128:(c4 + 4) * 128, 0:512]
                    .rearrange("(c p) j -> p c j", p=128))
                nc.sync.dma_start(
                    out=wq_sb[:, c4 * 512:(c4 + 4) * 512]
                    .rearrange("p (c j) -> p c j", j=512),
                    in_=wq_d[c4 * 128:(c4 + 4) * 128, :]
                    .rearrange("(c p) j -> p c j", p=128))
            for w in range(16):
                wps = ps_s.tile([128, 128], f32, tag="sps", name="wps")
                nc.tensor.matmul(wps[:], lhsT=warm_sb[:], rhs=warm_sb[:],
                                 start=True, stop=True)
            for t in kv0[16:]:            # V matmuls
                t()
            for w in range(16):
                wps = ps_s.tile([128, 128], f32, tag="sps", name="wps")
                nc.tensor.matmul(wps[:], lhsT=warm_sb[:], rhs=warm_sb[:],
                                 start=True, stop=True)
            _, q0 = q_chain_thunks(0, xt_tile=xt0)
            for t in q0:
                t()
            nc.sync.dma_start(
                out=wo_sb.rearrange("p (c j) -> p c j", j=2048),
                in_=wo_d.rearrange("(c p) j -> p c j", p=128))
            # fillers, dependency-safe order; record end index of each group
            chains = {}
            for i in (1, 2, 3):
                chains[f"kv{i}"] = kv_chain_thunks(i)
                chains[f"q{i}"] = q_chain_thunks(i)
            group_end = {}
            for name in ("kv1", "q1", "kv2", "q2", "kv3", "q3"):
                fillers.extend(chains[name][1])
                group_end[name] = len(fillers) - 1
            # stream chunk-1/2 inputs in right behind the prologue weights
            # (xs pool holds 4 buffers, so chunk 2 is prologue-safe)
            chains["kv1"][0]()
            chains["q1"][0]()
            chains["kv2"][0]()
            chains["q2"][0]()

            for jq in range(4):
                # producers attention(jq) needs must be emitted already
                if jq >= 1:
                    drain_fillers_through(group_end[f"kv{jq}"])
                    drain_fillers_through(group_end[f"q{jq}"])
                    # prefetch two blocks ahead now that the xs buffer
                    # being recycled has all its readers emitted
                    if jq + 2 <= 3:
                        chains[f"kv{jq + 2}"][0]()
                        chains[f"q{jq + 2}"][0]()
                pend = None
                for hc in range(4):
                    pend = attention_pair(hc, jq, qk0=pend)
                    pop_filler(2)
                # out-proj of this block becomes legal filler now
                fillers.extend(outproj_thunks(jq))
                group_end[f"op{jq}"] = len(fillers) - 1
            pop_filler(len(fillers))
    nc.finalize()
    return nc


def _get_nc():
    if "nc" not in _CACHE:
        _CACHE["nc"] = _build()
    return _CACHE["nc"]


def kernel(**inputs):
    out, _ = _run(inputs, trace=False)
    return out


def _run(inputs, trace=False):
    import ml_dtypes
    from concourse.bass_utils import run_bass_kernel_spmd

    x = np.asarray(inputs["x"], np.float32)
    kv = np.asarray(inputs["kv"], np.float32)
    Wq = np.asarray(inputs["Wq"], np.float32)
    bq = np.asarray(inputs["bq"], np.float32)
    Wk = np.asarray(inputs["Wk"], np.float32)
    bk = np.asarray(inputs["bk"], np.float32)
    Wv = np.asarray(inputs["Wv"], np.float32)
    bv = np.asarray(inputs["bv"], np.float32)
    Wo = np.asarray(inputs["Wo"], np.float32)
    bo = np.asarray(inputs["bo"], np.float32)

    bf = ml_dtypes.bfloat16
    M0 = (np.arange(1024)[None, :] >= (np.arange(128)[:, None] + 512)
          ).astype(bf)

    # head-dim permutation: chunk c = [local head c | local head 4+c]
    # so each head's Q rows sit at the partition half of its KV group.
    hperm = np.concatenate(
        [np.concatenate([np.arange(c * 64, c * 64 + 64),
                         np.arange((4 + c) * 64, (4 + c) * 64 + 64)])
         for c in range(4)])  # [512] permutation of local head dims

    in_maps = []
    for core in range(NCORES):
        b, t = core // 4, core % 4
        bv_sh = bv[t * 128:(t + 1) * 128]
        bvt = np.broadcast_to(bv_sh[None, :], (128, 128)).astype(np.float32)
        wq_sh = Wq[:, t * 512:(t + 1) * 512][:, hperm]
        wo_sh = Wo[t * 512:(t + 1) * 512, :][hperm, :]
        bq_sh = bq[t * 512:(t + 1) * 512][hperm]
        in_maps.append({
            "xT": np.ascontiguousarray(x[b].T).astype(bf),
            "kvT": np.ascontiguousarray(kv[b].T).astype(bf),
            "wq": wq_sh.astype(bf),
            "wk": Wk[:, t * 128:(t + 1) * 128].astype(bf),
            "wv": Wv[:, t * 128:(t + 1) * 128].astype(bf),
            "wo": np.ascontiguousarray(wo_sh).astype(bf),
            "bq": np.ascontiguousarray(bq_sh.reshape(4, 128).T),
            "bk": bk[t * 128:(t + 1) * 128].reshape(128, 1).copy(),
            "bvt": np.ascontiguousarray(bvt),
            "m0": M0,
        })

    nc = _get_nc()
    res = run_bass_kernel_spmd(nc, in_maps, core_ids=list(range(NCORES)),
                               trace=trace)
    parts = [np.asarray(res.results[i]["out"], np.float32)
             for i in range(NCORES)]
    out = np.stack([parts[0] + parts[1] + parts[2] + parts[3],
                    parts[4] + parts[5] + parts[6] + parts[7]])
    out += bo[None, None, :]
    return out.astype(np.float32), res


# revision 44
# speedup vs baseline: 1.0022x; 1.0022x over previous
"""GQA attention kernel for 8 TRN2 NeuronCores.

Sharding: data-parallel over batch (B=2) x tensor-parallel over heads (4-way).
Core i handles batch i//4 and head-shard i%4 (8 query heads = 2 KV groups).
Out-proj is row-sharded; the 4 partial [S,D] outputs per batch are summed on
the host (cheap unshard step), bo added once.

Device kernel (per core, all bf16 matmuls, f32 PSUM):
  QT = Wq_sh.T @ xT          [512, S]   (x pre-transposed on host)
  KT = Wk_sh.T @ kvT         [128, S]  -> stored zero-padded per group so QK
  runs with K=128 (full PE-array tile mode; avoids the 64-row mode-switch
  weight-load penalty)
  V  = kvT.T  @ Wv_sh        [S, 128] -> per-group V_aug [S, 64+1] (ones col)
  per (head-pair (hc, hc+4), q-chunk 512): scores^T chunks [128 keys, 512 q]
  for both heads off one shared rhs -> exp (no max subtraction; scores are
  O(1)) -> causal mask via sliding window of a precomputed [128,1024] 0/1
  mask -> PV accumulate with ones-row giving softmax sums in row 64 ->
  normalize both heads with ONE selector matmul broadcasting the two
  reciprocal rows -> OT [512, S] -> out_partial = OT.T @ Wo_sh  [S, D] f32.
"""

import numpy as np

B, S, D = 2, 2048, 2048
H, G, HD, GS = 32, 8, 64, 4
HPC = 8     # query heads per core
GPC = 2     # kv groups per core
NCORES = 8
SCALE = 0.125  # 1/sqrt(64)

_CACHE = {}


def _build():
    import concourse.bass as bass
    import concourse.tile as tile
    from concourse import bacc, mybir

    f32 = mybir.dt.float32
    bf16 = mybir.dt.bfloat16
    AF = mybir.ActivationFunctionType
    ALU = mybir.AluOpType

    nc = bacc.Bacc("TRN2", target_bir_lowering=False, debug=False,
                   num_devices=NCORES)

    xT_d = nc.declare_dram_parameter("xT", [D, S], bf16, isOutput=False)
    kvT_d = nc.declare_dram_parameter("kvT", [D, S], bf16, isOutput=False)
    wq_d = nc.declare_dram_parameter("wq", [D, 512], bf16, isOutput=False)
    wk_d = nc.declare_dram_parameter("wk", [D, 128], bf16, isOutput=False)
    wv_d = nc.declare_dram_parameter("wv", [D, 128], bf16, isOutput=False)
    wo_d = nc.declare_dram_parameter("wo", [512, D], bf16, isOutput=False)
    bq_d = nc.declare_dram_parameter("bq", [128, 4], f32, isOutput=False)
    bk_d = nc.declare_dram_parameter("bk", [128, 1], f32, isOutput=False)
    bvt_d = nc.declare_dram_parameter("bvt", [128, 2 * 64], f32, isOutput=False)
    m0_d = nc.declare_dram_parameter("m0", [128, 1024], bf16, isOutput=False)
    # partials are summed across 4 cores on the host in f32; bf16 partials
    # halve the output DMA traffic (the structural tail) for ~1e-3 extra err
    out_d = nc.declare_dram_parameter("out", [S, D], bf16, isOutput=True)

    with tile.TileContext(nc) as tc:
        with (
            tc.tile_pool(name="persist", bufs=1) as persist,
            tc.tile_pool(name="stream", bufs=4) as stream,
            tc.tile_pool(name="small", bufs=3) as small,
            tc.tile_pool(name="probs", bufs=8) as probs_pool,
            tc.tile_pool(name="ps_s", bufs=3, space="PSUM") as ps_s,
            tc.tile_pool(name="ps_proj", bufs=2, space="PSUM") as ps_proj,
            tc.tile_pool(name="ps_o", bufs=3, space="PSUM") as ps_o,
        ):
            # ---- resident weight tiles (DMAs emitted in compute order) ----
            wq_sb = persist.tile([128, 16 * 512], bf16, tag="wq")   # chunk c at c*512
            wk_sb = persist.tile([128, 16 * 128], bf16, tag="wk")
            wv_sb = persist.tile([128, 16 * 128], bf16, tag="wv")
            wo_sb = persist.tile([128, 4 * 2048], bf16, tag="wo")
            m0_sb = persist.tile([128, 1024], bf16, tag="m0")
            bq_sb = persist.tile([128, 4], f32, tag="bq")
            bk_sb = persist.tile([128, 1], f32, tag="bk")
            bvt_sb = persist.tile([128, 2 * 64], f32, tag="bvt")

            warm_sb = persist.tile([128, 128], bf16, tag="warm")
            nc.vector.memset(warm_sb[:], 0.25)
            for w in range(16):
                wps = ps_s.tile([128, 128], f32, tag="sps", name="wps")
                nc.tensor.matmul(wps[:], lhsT=warm_sb[:], rhs=warm_sb[:],
                                 start=True, stop=True)

            # programming a DMA costs ~650ns of serial Sync-engine time, but
            # the FIRST pieces gate the first matmuls: tiny head, fat tail
            for lo, hi in ((0, 1), (1, 16)):
                nc.sync.dma_start(
                    out=wk_sb[:, lo * 128:hi * 128]
                    .rearrange("p (c j) -> p c j", j=128),
                    in_=wk_d[lo * 128:hi * 128, :]
                    .rearrange("(c p) j -> p c j", p=128))

            sel_sb = persist.tile([128, 128], bf16, tag="sel")
            rsb2a = persist.tile([128, 512], bf16, tag="rsb2a")
            rsb2b = persist.tile([128, 512], bf16, tag="rsb2b")

            # ---- resident projection outputs ----
            qt_sb = persist.tile([128, 4 * 2048], bf16, tag="qt")   # chunk hc at hc*2048
            # K^T stored twice, zero-padded per group: ktp0 rows 0:64 = group0
            # K dims (rows 64:128 zero); ktp1 rows 64:128 = group1 (rows 0:64
            # zero). QK then contracts K=128 against the full qt partition
            # range; the zero rows kill the other head's contribution.
            ktp0_sb = persist.tile([128, S], bf16, tag="ktp0")
            ktp1_sb = persist.tile([128, S], bf16, tag="ktp1")
            vaug_sb = persist.tile([128, 2 * 16 * 65], bf16, tag="vaug")  # [gl*1040+tok*65]
            ot_sb = persist.tile([128, 4 * 2048], bf16, tag="ot")

            nc.vector.memset(ktp0_sb[64:128, :], 0.0)
            nc.vector.memset(ktp1_sb[0:64, :], 0.0)
            # selector for the paired reciprocal broadcast: out rows 0:64 get
            # rhs row 0, out rows 64:128 get rhs row 64
            nc.vector.memset(sel_sb[:], 0.0)
            nc.vector.memset(sel_sb[0:1, 0:64], 1.0)
            nc.vector.memset(sel_sb[64:65, 64:128], 1.0)
            # only rows 0 and 64 are ever written after this -> no garbage
            nc.vector.memset(rsb2a[:], 0.0)
            nc.vector.memset(rsb2b[:], 0.0)

            # ---- chain emitters (as thunk lists for PE-filler interleave) ----
            def kv_chain_thunks(tch):
                """K/V projection for kv token chunk tch.

                Returns (dma_fn, compute_thunks): the DMA is emitted eagerly
                (a block ahead) so filler matmuls never starve on it."""
                th = []
                state = {}

                def dma():
                    kvt = stream.tile([128, 16 * 512], bf16, tag="xs", name="kvt")
                    pieces = ((0, 1), (1, 4), (4, 16)) if tch == 0 else \
                             ((0, 8), (8, 16))
                    for lo, hi in pieces:
                        nc.sync.dma_start(
                            out=kvt[:, lo * 512:hi * 512]
                            .rearrange("p (c j) -> p c j", j=512),
                            in_=kvT_d[lo * 128:hi * 128,
                                      tch * 512:(tch + 1) * 512]
                            .rearrange("(c p) j -> p c j", p=128))
                    if tch == 0:
                        nc.sync.dma_start(out=bk_sb[:], in_=bk_d[:, :])
                        nc.sync.dma_start(out=bvt_sb[:], in_=bvt_d[:, :])
                    state["kvt"] = kvt

                def kmm(c):
                    if c == 0:
                        state["kps"] = ps_proj.tile([128, 512], f32, tag="proj",
                                                    name="kps")
                    nc.tensor.matmul(
                        state["kps"][:], lhsT=wk_sb[:, c * 128:(c + 1) * 128],
                        rhs=state["kvt"][:, c * 512:(c + 1) * 512],
                        start=(c == 0), stop=(c == 15))
                    if c == 15:
                        win = slice(tch * 512, (tch + 1) * 512)
                        nc.vector.tensor_scalar(
                            ktp0_sb[0:64, win], state["kps"][0:64, :],
                            bk_sb[0:64, 0:1], None, ALU.add)
                        nc.vector.tensor_scalar(
                            ktp1_sb[64:128, win], state["kps"][64:128, :],
                            bk_sb[64:128, 0:1], None, ALU.add)
                for c in range(16):
                    th.append(lambda c=c: kmm(c))

                def vmm(tt, c):
                    if c == 0:
                        state["vps"] = ps_proj.tile([128, 128], f32, tag="proj",
                                                    name="vps")
                    nc.tensor.matmul(
                        state["vps"][:],
                        lhsT=state["kvt"][:, c * 512 + tt * 128:
                                          c * 512 + (tt + 1) * 128],
                        rhs=wv_sb[:, c * 128:(c + 1) * 128],
                        start=(c == 0), stop=(c == 15))
                    if c == 15:
                        tok = tch * 4 + tt
                        for gl in range(2):
                            base = gl * 1040 + tok * 65
                            nc.vector.tensor_tensor(
                                vaug_sb[:, base:base + 64],
                                state["vps"][:, gl * 64:(gl + 1) * 64],
                                bvt_sb[:, gl * 64:(gl + 1) * 64], ALU.add)
                            nc.vector.memset(
                                vaug_sb[:, base + 64:base + 65], 1.0)
                for tt in range(4):
                    for c in range(0, 16, 4):
                        # 4 small matmuls per thunk (they are ~68ns each)
                        def v4(tt=tt, c0=c):
                            for c in range(c0, c0 + 4):
                                vmm(tt, c)
                        th.append(v4)
                return dma, th

            def q_chain_thunks(qch, xt_tile=None):
                """Q projection for q chunk qch: (dma_fn, compute thunks)."""
                th = []
                state = {}

                if xt_tile is not None:
                    state["xt"] = xt_tile
                    dma = None
                else:
                    def dma():
                        xt = stream.tile([128, 16 * 512], bf16, tag="xs",
                                         name="xt")
                        nc.sync.dma_start(
                            out=xt.rearrange("p (c j) -> p c j", j=512),
                            in_=xT_d[:, qch * 512:(qch + 1) * 512]
                            .rearrange("(c p) j -> p c j", p=128))
                        state["xt"] = xt

                def qmm(hc, c):
                    if c == 0:
                        state["qps"] = ps_proj.tile([128, 512], f32, tag="proj",
                                                    name="qps")
                    nc.tensor.matmul(
                        state["qps"][:],
                        lhsT=wq_sb[:, c * 512 + hc * 128:c * 512 + (hc + 1) * 128],
                        rhs=state["xt"][:, c * 512:(c + 1) * 512],
                        start=(c == 0), stop=(c == 15))
                    if c == 15:
                        nc.vector.tensor_scalar(
                            qt_sb[:, hc * 2048 + qch * 512:
                                  hc * 2048 + (qch + 1) * 512],
                            state["qps"][:], bq_sb[:, hc:hc + 1], None, ALU.add)
                for hc in range(4):
                    for c in range(16):
                        th.append(lambda hc=hc, c=c: qmm(hc, c))
                return dma, th

            def outproj_thunks(jqb):
                """Out-projection for q block jqb (4 q-tiles x 4 col-chunks)."""
                th = []
                state = {}

                def omm(qt_i, cc, c):
                    if c == 0:
                        state["outp"] = ps_proj.tile([128, 512], f32, tag="proj",
                                                     name="outp")
                    nc.tensor.matmul(
                        state["outp"][:],
                        lhsT=ot_sb[:, c * 2048 + qt_i * 128:
                                   c * 2048 + (qt_i + 1) * 128],
                        rhs=wo_sb[:, c * 2048 + cc * 512:c * 2048 + (cc + 1) * 512],
                        start=(c == 0), stop=(c == 3))
                    if c == 3:
                        if cc == 0:
                            state["osb"] = stream.tile([128, 2048], bf16,
                                                       tag="osb", name="osb")
                        # scalar engine: idle outside the exp stream
                        nc.scalar.activation(
                            state["osb"][:, cc * 512:(cc + 1) * 512],
                            state["outp"][:], AF.Copy)
                        if cc == 3:
                            nc.sync.dma_start(
                                out=out_d[qt_i * 128:(qt_i + 1) * 128, :],
                                in_=state["osb"][:])
                for qt_i in range(jqb * 4, jqb * 4 + 4):
                    for cc in range(4):
                        for c in range(4):
                            th.append(lambda q=qt_i, cc=cc, c=c: omm(q, cc, c))
                return th

            # ---- filler queue machinery ----
            fillers = []
            fpos = [0]

            def pop_filler(n=1):
                while n > 0 and fpos[0] < len(fillers):
                    fillers[fpos[0]]()
                    fpos[0] += 1
                    n -= 1

            def drain_fillers_through(idx):
                while fpos[0] <= idx:
                    fillers[fpos[0]]()
                    fpos[0] += 1

            def qk0_pair(hc, jq):
                # next pair's first QK scores, pre-emitted before the current
                # pair's final PV so exp(0) overlaps the norm chain
                qbase = hc * 2048 + jq * 512
                rhs = qt_sb[:, qbase:qbase + 512]
                spsA = ps_s.tile([128, 512], f32, tag="sps", name="spsA")
                nc.tensor.matmul(spsA[:], lhsT=ktp0_sb[:, 0:128], rhs=rhs,
                                 start=True, stop=True)
                spsB = ps_s.tile([128, 512], f32, tag="sps", name="spsB")
                nc.tensor.matmul(spsB[:], lhsT=ktp1_sb[:, 0:128], rhs=rhs,
                                 start=True, stop=True)
                return (spsA, spsB, 0)

            # ---- attention for one head-pair (hc, hc+4) on one q-chunk ----
            def attention_pair(hc, jq, qk0=None):
                nkc = 4 * jq + 4
                qbase = hc * 2048 + jq * 512
                opsA = ps_o.tile([65, 512], f32, tag="ops", name="opsA")
                opsB = ps_o.tile([65, 512], f32, tag="ops", name="opsB")
                sps_t = {}
                pt_t = {}

                def emit_qk(kci):
                    m = max(0, kci * 128 - jq * 512)
                    rhs = qt_sb[:, qbase + m:qbase + 512]
                    spsA = ps_s.tile([128, 512], f32, tag="sps", name="spsA")
                    nc.tensor.matmul(
                        spsA[:, m:512],
                        lhsT=ktp0_sb[:, kci * 128:(kci + 1) * 128],
                        rhs=rhs, start=True, stop=True)
                    spsB = ps_s.tile([128, 512], f32, tag="sps", name="spsB")
                    nc.tensor.matmul(
                        spsB[:, m:512],
                        lhsT=ktp1_sb[:, kci * 128:(kci + 1) * 128],
                        rhs=rhs, start=True, stop=True)
                    sps_t[kci] = (spsA, spsB, m)

                def emit_exp(kci):
                    spsA, spsB, m = sps_t.pop(kci)
                    ptA = probs_pool.tile([128, 512], bf16, tag="pt", name="ptA")
                    nc.scalar.activation(ptA[:, m:512], spsA[:, m:512],
                                         AF.Exp, scale=SCALE)
                    ptB = probs_pool.tile([128, 512], bf16, tag="pt", name="ptB")
                    nc.scalar.activation(ptB[:, m:512], spsB[:, m:512],
                                         AF.Exp, scale=SCALE)
                    if kci >= 4 * jq:   # diagonal chunk -> mask
                        nc.vector.tensor_tensor(
                            ptA[:, m:512], ptA[:, m:512],
                            m0_sb[:, 512:1024 - m], ALU.mult)
                        nc.vector.tensor_tensor(
                            ptB[:, m:512], ptB[:, m:512],
                            m0_sb[:, 512:1024 - m], ALU.mult)
                    pt_t[kci] = (ptA, ptB, m)

                def emit_pv(kci):
                    ptA, ptB, m = pt_t.pop(kci)
                    nc.tensor.matmul(
                        opsA[:, m:512],
                        lhsT=vaug_sb[:, kci * 65:kci * 65 + 65],
                        rhs=ptA[:, m:512],
                        start=(kci == 0), stop=(kci == nkc - 1))
                    nc.tensor.matmul(
                        opsB[:, m:512],
                        lhsT=vaug_sb[:, 1040 + kci * 65:1040 + kci * 65 + 65],
                        rhs=ptB[:, m:512],
                        start=(kci == 0), stop=(kci == nkc - 1))

                rss2 = small.tile([33, 512], f32, tag="rss", name="rss2")

                def emit_pv_last(kci):
                    # final PV pair: tuck each head's sums copy right behind
                    # its matmul so the vector norm chain starts early
                    ptA, ptB, m = pt_t.pop(kci)
                    nc.tensor.matmul(
                        opsA[:, m:512],
                        lhsT=vaug_sb[:, kci * 65:kci * 65 + 65],
                        rhs=ptA[:, m:512], start=(kci == 0), stop=True)
                    nc.vector.tensor_copy(rss2[0:1, :], opsA[64:65, :])
                    nc.tensor.matmul(
                        opsB[:, m:512],
                        lhsT=vaug_sb[:, 1040 + kci * 65:1040 + kci * 65 + 65],
                        rhs=ptB[:, m:512], start=(kci == 0), stop=True)
                    nc.vector.tensor_copy(rss2[32:33, :], opsB[64:65, :])

                if qk0 is None:
                    emit_qk(0)
                else:
                    sps_t[0] = qk0
                nxt = None
                for kci in range(nkc):
                    emit_exp(kci)
                    if kci + 1 < nkc:
                        emit_qk(kci + 1)
                    pop_filler(3 if jq <= 1 else 2)
                    if kci < nkc - 1:
                        emit_pv(kci)
                    else:
                        if hc < 3:
                            nxt = qk0_pair(hc + 1, jq)
                        emit_pv_last(kci)

                # paired normalize: two reciprocal rows -> one K=128 selector
                # matmul broadcasts row 0 to out rows 0:64, row 64 to 64:128
                rsb2 = rsb2a if hc % 2 == 0 else rsb2b
                rs2 = small.tile([33, 512], f32, tag="rs", name="rs2")
                nc.vector.reciprocal_approx_fast(rs2[:], rss2[:])
                nc.vector.tensor_copy(rsb2[0:1, :], rs2[0:1, :])
                nc.vector.tensor_copy(rsb2[64:65, :], rs2[32:33, :])
                if hc == 3 and jq == 3:
                    # keep ps_proj free for the trailing out-proj chains:
                    # the sps pool is idle once the last scores are consumed
                    bps = ps_s.tile([128, 512], f32, tag="sps", name="bps")
                else:
                    bps = ps_proj.tile([128, 512], f32, tag="proj", name="bps")
                nc.tensor.matmul(bps[:], lhsT=sel_sb[:], rhs=rsb2[:],
                                 start=True, stop=True)
                bsb = small.tile([128, 512], f32, tag="bsb", name="bsb")
                nc.vector.tensor_copy(bsb[:], bps[:])
                nc.vector.tensor_tensor(
                    ot_sb[0:64, qbase:qbase + 512],
                    opsA[0:64, :], bsb[0:64, :], ALU.mult)
                nc.vector.tensor_tensor(
                    ot_sb[64:128, qbase:qbase + 512],
                    opsB[0:64, :], bsb[64:128, :], ALU.mult)
                return nxt

            # ---- emission schedule ----
            # prologue: KV(0) K-chain first (gated only on wk piece 0 + kvt
            # piece 0), then Q(0); remaining weight DMAs stream in behind
            kv0_dma, kv0 = kv_chain_thunks(0)
            kv0_dma()                     # kvt0 DMA pieces + bk + bvt
            for t in kv0[:16]:            # 16 K matmuls
                t()
            nc.sync.dma_start(out=bq_sb[:], in_=bq_d[:, :])
            nc.sync.dma_start(
                out=wv_sb.rearrange("p (c j) -> p c j", j=128),
                in_=wv_d.rearrange("(c p) j -> p c j", p=128))
            nc.sync.dma_start(out=m0_sb[:], in_=m0_d[:, :])
            # xt0 + wq interleaved in 2 pieces each: Q chain c=0 needs only
            # piece 0 of both
            xt0 = stream.tile([128, 16 * 512], bf16, tag="xs", name="xt0")
            for c4 in range(0, 16, 8):
                nc.sync.dma_start(
                    out=xt0[:, c4 * 512:(c4 + 8) * 512]
                    .rearrange("p (c j) -> p c j", j=512),
                    in_=xT_d[c4 * 128:(c4 + 8) * 128, 0:512]
                    .rearrange("(c p) j -> p c j", p=128))
                nc.sync.dma_start(
                    out=wq_sb[:, c4 * 512:(c4 + 8) * 512]
                    .rearrange("p (c j) -> p c j", j=512),
                    in_=wq_d[c4 * 128:(c4 + 8) * 128, :]
                    .rearrange("(c p) j -> p c j", p=128))
            for t in kv0[16:]:            # V matmuls
                t()
            _, q0 = q_chain_thunks(0, xt_tile=xt0)
            for t in q0:
                t()
            nc.sync.dma_start(
                out=wo_sb.rearrange("p (c j) -> p c j", j=2048),
                in_=wo_d.rearrange("(c p) j -> p c j", p=128))
            # fillers, dependency-safe order; record end index of each group
            chains = {}
            for i in (1, 2, 3):
                chains[f"kv{i}"] = kv_chain_thunks(i)
                chains[f"q{i}"] = q_chain_thunks(i)
            group_end = {}
            for name in ("kv1", "q1", "kv2", "q2", "kv3", "q3"):
                fillers.extend(chains[name][1])
                group_end[name] = len(fillers) - 1
            # stream chunk-1/2 inputs in right behind the prologue weights
            # (xs pool holds 4 buffers, so chunk 2 is prologue-safe)
            chains["kv1"][0]()
            chains["q1"][0]()
            chains["kv2"][0]()
            chains["q2"][0]()

            for jq in range(4):
                # producers attention(jq) needs must be emitted already
                if jq >= 1:
                    drain_fillers_through(group_end[f"kv{jq}"])
                    drain_fillers_through(group_end[f"q{jq}"])
                    # prefetch two blocks ahead now that the xs buffer
                    # being recycled has all its readers emitted
                    if jq + 2 <= 3:
                        chains[f"kv{jq + 2}"][0]()
                        chains[f"q{jq + 2}"][0]()
                pend = None
                for hc in range(4):
                    pend = attention_pair(hc, jq, qk0=pend)
                    pop_filler(2)
                # out-proj of this block becomes legal filler now
                fillers.extend(outproj_thunks(jq))
                group_end[f"op{jq}"] = len(fillers) - 1
            pop_filler(len(fillers))
    nc.finalize()
    return nc


def _get_nc():
    if "nc" not in _CACHE:
        _CACHE["nc"] = _build()
    return _CACHE["nc"]


def kernel(**inputs):
    out, _ = _run(inputs, trace=False)
    return out


def _run(inputs, trace=False):
    import ml_dtypes
    from concourse.bass_utils import run_bass_kernel_spmd

    x = np.asarray(inputs["x"], np.float32)
    kv = np.asarray(inputs["kv"], np.float32)
    Wq = np.asarray(inputs["Wq"], np.float32)
    bq = np.asarray(inputs["bq"], np.float32)
    Wk = np.asarray(inputs["Wk"], np.float32)
    bk = np.asarray(inputs["bk"], np.float32)
    Wv = np.asarray(inputs["Wv"], np.float32)
    bv = np.asarray(inputs["bv"], np.float32)
    Wo = np.asarray(inputs["Wo"], np.float32)
    bo = np.asarray(inputs["bo"], np.float32)

    bf = ml_dtypes.bfloat16
    M0 = (np.arange(1024)[None, :] >= (np.arange(128)[:, None] + 512)
          ).astype(bf)

    # head-dim permutation: chunk c = [local head c | local head 4+c]
    # so each head's Q rows sit at the partition half of its KV group.
    hperm = np.concatenate(
        [np.concatenate([np.arange(c * 64, c * 64 + 64),
                         np.arange((4 + c) * 64, (4 + c) * 64 + 64)])
         for c in range(4)])  # [512] permutation of local head dims

    in_maps = []
    for core in range(NCORES):
        b, t = core // 4, core % 4
        bv_sh = bv[t * 128:(t + 1) * 128]
        bvt = np.broadcast_to(bv_sh[None, :], (128, 128)).astype(np.float32)
        wq_sh = Wq[:, t * 512:(t + 1) * 512][:, hperm]
        wo_sh = Wo[t * 512:(t + 1) * 512, :][hperm, :]
        bq_sh = bq[t * 512:(t + 1) * 512][hperm]
        in_maps.append({
            "xT": np.ascontiguousarray(x[b].T).astype(bf),
            "kvT": np.ascontiguousarray(kv[b].T).astype(bf),
            "wq": wq_sh.astype(bf),
            "wk": Wk[:, t * 128:(t + 1) * 128].astype(bf),
            "wv": Wv[:, t * 128:(t + 1) * 128].astype(bf),
            "wo": np.ascontiguousarray(wo_sh).astype(bf),
            "bq": np.ascontiguousarray(bq_sh.reshape(4, 128).T),
            "bk": bk[t * 128:(t + 1) * 128].reshape(128, 1).copy(),
            "bvt": np.ascontiguousarray(bvt),
            "m0": M0,
        })

    nc = _get_nc()
    res = run_bass_kernel_spmd(nc, in_maps, core_ids=list(range(NCORES)),
                               trace=trace)
    parts = [np.asarray(res.results[i]["out"], np.float32)
             for i in range(NCORES)]
    out = np.stack([parts[0] + parts[1] + parts[2] + parts[3],
                    parts[4] + parts[5] + parts[6] + parts[7]])
    out += bo[None, None, :]
    return out.astype(np.float32), res
